# revision 3
# baseline (speedup 1.0000x reference)
"""Trainium2 Bass kernel: Attractor fixed-point iteration.

Reference math (fp32):
    x:[16,4096,256] -> flatten rows R=65536
    c = x @ W_in.T + b_in                     (R, 512)
    Ws = 0.5*(W + W.T)      (symmetric => a @ Ws.T == a @ Ws)
    a_{k+1} = tanh(a_k @ Ws + b + c),  a_0 = 0, 15 iterations
    y = a_15 @ W_out.T + b_out                (R, 256) -> [16,4096,256]

Mapping: data-parallel over rows across 8 NeuronCores (8192 rows/core),
weights replicated (per spec sharding hint).  Per core, rows are
processed in tiles of 512; activations live feature-partitioned in SBUF
as [128 part=feature, chunk, row].  The recurrent matmuls run in bf16
(fp32 PSUM accumulation); since a_0 = 0, iteration 1 reduces to
a_1 = tanh(c + bias) and is fused with the input projection.  The in/out
projections use a 3-term bf16 hi/lo split (x_hi@W_hi + x_lo@W_hi +
x_hi@W_lo) because the identity-dominated projection weights and the
drive term c feed the output directly; measured end-to-end
absmax/scale ~6e-4 vs the fp32 reference.

Host side: x is transposed/split per core into bf16 hi/lo feature-major
layout; the kernel emits y transposed ([feat, row]) and the host
transposes back and adds b_out.
"""

import numpy as np
import ml_dtypes

import concourse.bass as bass
import concourse.mybir as mybir
import concourse.tile as tile
from concourse import bacc
from concourse import bass_utils

BF16 = ml_dtypes.bfloat16
F32 = mybir.dt.float32
BF = mybir.dt.bfloat16
TANH = mybir.ActivationFunctionType.Tanh

B, L, C = 16, 4096, 256
N = 512
K_ITERS = 15
N_CORES = 8
R_TOT = B * L                 # 65536
R_CORE = R_TOT // N_CORES     # 8192
TILE_R = 512
JC = N // 128                 # 4 hidden-feature chunks
MC = C // 128                 # 2 channel chunks


def _body(tc, ins, yt, r_core):
    nc = tc.nc
    ntiles = r_core // TILE_R
    assert ntiles % 2 == 0
    with (
        tc.tile_pool(name="wpool", bufs=1) as wpool,
        tc.tile_pool(name="xpool", bufs=4) as xpool,
        tc.tile_pool(name="cpool", bufs=3) as cpool,
        tc.tile_pool(name="apool", bufs=6) as apool,
        tc.tile_pool(name="tpool", bufs=4) as tpool,
        tc.tile_pool(name="opool", bufs=2) as opool,
        tc.tile_pool(name="ypool", bufs=3) as ypool,
        tc.tile_pool(name="zpool", bufs=2, space="PSUM") as zpool,
    ):
        # ---- resident weights ----
        ws_sb = wpool.tile([128, JC, JC, 128], BF, tag="ws")
        for ic in range(JC):
            nc.sync.dma_start(ws_sb[:, ic, :, :], ins["ws"][ic])
        wi_hi = wpool.tile([128, MC, JC, 128], BF, tag="wi_hi")
        wi_lo = wpool.tile([128, MC, JC, 128], BF, tag="wi_lo")
        for mc in range(MC):
            nc.sync.dma_start(wi_hi[:, mc, :, :], ins["wi_hi"][mc])
            nc.sync.dma_start(wi_lo[:, mc, :, :], ins["wi_lo"][mc])
        wo_hi = wpool.tile([128, JC, MC, 128], BF, tag="wo_hi")
        wo_lo = wpool.tile([128, JC, MC, 128], BF, tag="wo_lo")
        for jc in range(JC):
            nc.sync.dma_start(wo_hi[:, jc, :, :], ins["wo_hi"][jc])
            nc.sync.dma_start(wo_lo[:, jc, :, :], ins["wo_lo"][jc])
        bias_sb = wpool.tile([128, JC, 1], F32, tag="bias")
        for jc in range(JC):
            nc.sync.dma_start(bias_sb[:, jc, :], ins["bias"][jc])

        # ---- row tiles, processed in interleaved pairs ----
        for tp in range(ntiles // 2):
            ctx = []
            for t in (2 * tp, 2 * tp + 1):
                xh = xpool.tile([128, MC, TILE_R], BF, tag="xh")
                xl = xpool.tile([128, MC, TILE_R], BF, tag="xl")
                for mc in range(MC):
                    nc.sync.dma_start(
                        xh[:, mc, :], ins["xt_hi"][mc, :, bass.ts(t, TILE_R)]
                    )
                    nc.sync.dma_start(
                        xl[:, mc, :], ins["xt_lo"][mc, :, bass.ts(t, TILE_R)]
                    )
                zps = zpool.tile([128, JC, TILE_R], F32, tag="z")
                ctx.append(dict(t=t, xh=xh, xl=xl, zps=zps))

            # input projection: c = x @ W_in.T  (3-term hi/lo, bf16 matmuls)
            for d in ctx:
                zps = d["zps"]
                for jc in range(JC):
                    mmi = 0
                    for w, xx in (
                        (wi_hi, d["xh"]),
                        (wi_hi, d["xl"]),
                        (wi_lo, d["xh"]),
                    ):
                        for mc in range(MC):
                            nc.tensor.matmul(
                                zps[:, jc, :],
                                w[:, mc, jc, :],
                                xx[:, mc, :],
                                start=(mmi == 0),
                                stop=(mmi == 3 * MC - 1),
                            )
                            mmi += 1

            # keep c in SBUF; iteration 1: a_1 = tanh(c + bias)  (a_0 = 0)
            for d in ctx:
                c_sb = cpool.tile([128, JC, TILE_R], F32, tag="c")
                a = apool.tile([128, JC, TILE_R], BF, tag="a")
                for jc in range(JC):
                    nc.vector.tensor_copy(c_sb[:, jc, :], d["zps"][:, jc, :])
                    nc.scalar.activation(
                        a[:, jc, :], d["zps"][:, jc, :], TANH,
                        bias=bias_sb[:, jc, :],
                    )
                d["c"] = c_sb
                d["a"] = a

            # iterations 2..15
            for k in range(1, K_ITERS):
                last = k == K_ITERS - 1
                for d in ctx:
                    zps, a = d["zps"], d["a"]
                    for ic in range(JC):
                        for jc in range(JC):
                            nc.tensor.matmul(
                                zps[:, jc, :],
                                ws_sb[:, ic, jc, :],
                                a[:, ic, :],
                                start=(ic == 0),
                                stop=(ic == JC - 1),
                            )
                for d in ctx:
                    t_sb = tpool.tile([128, JC, TILE_R], F32, tag="t")
                    a_new = apool.tile([128, JC, TILE_R], BF, tag="a")
                    af = (
                        opool.tile([128, JC, TILE_R], F32, tag="af", name="af")
                        if last
                        else None
                    )
                    for jc in range(JC):
                        nc.vector.tensor_add(
                            t_sb[:, jc, :], d["zps"][:, jc, :], d["c"][:, jc, :]
                        )
                        nc.scalar.activation(
                            a_new[:, jc, :], t_sb[:, jc, :], TANH,
                            bias=bias_sb[:, jc, :],
                        )
                        if last:
                            nc.scalar.activation(
                                af[:, jc, :], t_sb[:, jc, :], TANH,
                                bias=bias_sb[:, jc, :],
                            )
                    d["a"] = a_new
                    if last:
                        d["af"] = af

            # a_lo = bf16(a_f32 - a_bf16) for the split output projection
            for d in ctx:
                a_lo = opool.tile([128, JC, TILE_R], BF, tag="alo")
                for jc in range(JC):
                    nc.vector.tensor_sub(
                        a_lo[:, jc, :], d["af"][:, jc, :], d["a"][:, jc, :]
                    )
                d["alo"] = a_lo

            # output projection: yT = W_out @ a  (3-term hi/lo), reusing
            # the first MC banks of the (now closed) z PSUM tile.
            for d in ctx:
                zps = d["zps"]
                for mc in range(MC):
                    mmi = 0
                    for w, aa in (
                        (wo_hi, d["a"]),
                        (wo_hi, d["alo"]),
                        (wo_lo, d["a"]),
                    ):
                        for jc in range(JC):
                            nc.tensor.matmul(
                                zps[:, mc, :],
                                w[:, jc, mc, :],
                                aa[:, jc, :],
                                start=(mmi == 0),
                                stop=(mmi == 3 * JC - 1),
                            )
                            mmi += 1
            for d in ctx:
                y_sb = ypool.tile([128, MC, TILE_R], F32, tag="y")
                for mc in range(MC):
                    nc.vector.tensor_copy(y_sb[:, mc, :], d["zps"][:, mc, :])
                    nc.sync.dma_start(
                        yt[mc, :, bass.ts(d["t"], TILE_R)], y_sb[:, mc, :]
                    )


def build_program(r_core=R_CORE, enable_asserts=False):
    nc = bacc.Bacc(
        "TRN2",
        target_bir_lowering=False,
        debug=False,
        enable_asserts=enable_asserts,
        num_devices=N_CORES,
        enable_partition_id=False,
    )
    ins = {
        "xt_hi": nc.dram_tensor(
            "xt_hi", [MC, 128, r_core], BF, kind="ExternalInput"
        ).ap(),
        "xt_lo": nc.dram_tensor(
            "xt_lo", [MC, 128, r_core], BF, kind="ExternalInput"
        ).ap(),
        "ws": nc.dram_tensor(
            "ws", [JC, 128, JC, 128], BF, kind="ExternalInput"
        ).ap(),
        "wi_hi": nc.dram_tensor(
            "wi_hi", [MC, 128, JC, 128], BF, kind="ExternalInput"
        ).ap(),
        "wi_lo": nc.dram_tensor(
            "wi_lo", [MC, 128, JC, 128], BF, kind="ExternalInput"
        ).ap(),
        "wo_hi": nc.dram_tensor(
            "wo_hi", [JC, 128, MC, 128], BF, kind="ExternalInput"
        ).ap(),
        "wo_lo": nc.dram_tensor(
            "wo_lo", [JC, 128, MC, 128], BF, kind="ExternalInput"
        ).ap(),
        "bias": nc.dram_tensor(
            "bias", [JC, 128, 1], F32, kind="ExternalInput"
        ).ap(),
    }
    yt = nc.dram_tensor(
        "yt", [MC, 128, r_core], F32, kind="ExternalOutput"
    ).ap()

    with tile.TileContext(nc) as tc:
        _body(tc, ins, yt, r_core)
    nc.compile()
    return nc


def _hi_lo(a):
    hi = a.astype(BF16)
    lo = (a - hi.astype(np.float32)).astype(BF16)
    return hi, lo


def prep_in_maps(x, W_in, b_in, W, b, W_out, b_out, r_core=R_CORE, n_cores=N_CORES):
    """Host-side packing: weight transposes/splits + per-core x shards."""
    x = np.ascontiguousarray(np.asarray(x, np.float32)).reshape(-1, C)
    W_in = np.asarray(W_in, np.float32)
    W = np.asarray(W, np.float32)
    W_out = np.asarray(W_out, np.float32)

    Ws = 0.5 * (W + W.T)
    ws_pk = Ws.astype(BF16).reshape(JC, 128, JC, 128)

    wi_hi, wi_lo = _hi_lo(np.ascontiguousarray(W_in.T))      # [C, N]
    wi_hi = wi_hi.reshape(MC, 128, JC, 128)
    wi_lo = wi_lo.reshape(MC, 128, JC, 128)

    wo_hi, wo_lo = _hi_lo(np.ascontiguousarray(W_out.T))     # [N, C]
    wo_hi = wo_hi.reshape(JC, 128, MC, 128)
    wo_lo = wo_lo.reshape(JC, 128, MC, 128)

    bias = (np.asarray(b, np.float32) + np.asarray(b_in, np.float32)).reshape(
        JC, 128, 1
    )

    shared = {
        "ws": ws_pk,
        "wi_hi": wi_hi,
        "wi_lo": wi_lo,
        "wo_hi": wo_hi,
        "wo_lo": wo_lo,
        "bias": bias,
    }
    in_maps = []
    for core in range(n_cores):
        xt = np.ascontiguousarray(
            x[core * r_core : (core + 1) * r_core].T
        )                                                    # [C, r_core]
        xt_hi, xt_lo = _hi_lo(xt)
        m = dict(shared)
        m["xt_hi"] = xt_hi.reshape(MC, 128, r_core)
        m["xt_lo"] = xt_lo.reshape(MC, 128, r_core)
        in_maps.append(m)
    return in_maps


def assemble_output(results, b_out, r_core=R_CORE):
    """results: list of per-core {"yt": [MC,128,r_core] f32} -> [B,L,C]."""
    parts = []
    for res in results:
        yt = np.asarray(res["yt"], np.float32).reshape(C, r_core)
        parts.append(yt.T)
    y = np.concatenate(parts, axis=0)
    y = y + np.asarray(b_out, np.float32)[None, :]
    if y.shape[0] == R_TOT:
        y = y.reshape(B, L, C)
    return np.ascontiguousarray(y.astype(np.float32))


_PROGRAM = None


def get_program():
    global _PROGRAM
    if _PROGRAM is None:
        _PROGRAM = build_program()
    return _PROGRAM


def run(inputs, trace=False, trace_kwargs=None):
    """Compile (cached) + execute on 8 cores; returns BassKernelResults."""
    nc = get_program()
    in_maps = prep_in_maps(**inputs)
    res = bass_utils.run_bass_kernel_spmd(
        nc,
        in_maps,
        core_ids=list(range(N_CORES)),
        trace=trace,
        **(trace_kwargs or {}),
    )
    return res


def kernel(x, W_in, b_in, W, b, W_out, b_out):
    inputs = dict(
        x=x, W_in=W_in, b_in=b_in, W=W, b=b, W_out=W_out, b_out=b_out
    )
    res = run(inputs, trace=False)
    return assemble_output(res.results, b_out)


# revision 5
# speedup vs baseline: 1.0274x; 1.0274x over previous
"""Trainium2 Bass kernel: Attractor fixed-point iteration.

Reference math (fp32):
    x:[16,4096,256] -> flatten rows R=65536
    c = x @ W_in.T + b_in                     (R, 512)
    Ws = 0.5*(W + W.T)      (symmetric => a @ Ws.T == a @ Ws)
    a_{k+1} = tanh(a_k @ Ws + b + c),  a_0 = 0, 15 iterations
    y = a_15 @ W_out.T + b_out                (R, 256) -> [16,4096,256]

Mapping: data-parallel over rows across 8 NeuronCores (8192 rows/core),
weights replicated (per spec sharding hint).  Per core, rows are
processed in tiles of 512; activations live feature-partitioned in SBUF
as [128 part=feature, chunk, row].  All matmuls run as float32r (fp32
bits through the PE at full 1 cycle/row rate for moving dim >= 256;
HW-probed accuracy ~1.8e-4 relmax per 128-contraction vs 2.6e-3 for
bf16), accumulating fp32 in PSUM.  Since a_0 = 0, iteration 1 reduces
to a_1 = tanh(c + bias) and is fused with the input projection.  Row
tiles are processed in interleaved pairs (PSUM holds 2 x 4 banks) so
the tensor engine stays busy while DVE adds c and ACT applies tanh.

Host side: x is transposed per core into feature-major [C, rows] fp32;
the kernel emits y transposed ([C, rows]) and the host transposes back
and adds b_out.
"""

import numpy as np

import concourse.bass as bass
import concourse.mybir as mybir
import concourse.tile as tile
from concourse import bacc
from concourse import bass_utils

F32 = mybir.dt.float32
F32R = mybir.dt.float32r
TANH = mybir.ActivationFunctionType.Tanh

B, L, C = 16, 4096, 256
N = 512
K_ITERS = 15
N_CORES = 8
R_TOT = B * L                 # 65536
R_CORE = R_TOT // N_CORES     # 8192
TILE_R = 512
JC = N // 128                 # 4 hidden-feature chunks
MC = C // 128                 # 2 channel chunks


def _mm(nc, out, lhsT, rhs, start, stop):
    nc.tensor.matmul(out, lhsT, rhs, start=start, stop=stop)


def _body(tc, ins, yt, r_core):
    nc = tc.nc
    ntiles = r_core // TILE_R
    assert ntiles % 2 == 0
    with (
        tc.tile_pool(name="wpool", bufs=1) as wpool,
        tc.tile_pool(name="xpool", bufs=4) as xpool,
        tc.tile_pool(name="cpool", bufs=3) as cpool,
        tc.tile_pool(name="apool", bufs=6) as apool,
        tc.tile_pool(name="tpool", bufs=4) as tpool,
        tc.tile_pool(name="ypool", bufs=3) as ypool,
        tc.tile_pool(name="zpool", bufs=2, space="PSUM") as zpool,
    ):
        # ---- resident weights (fp32, bitcast to f32r at matmul sites) ----
        ws_sb = wpool.tile([128, JC, JC, 128], F32R, tag="ws")
        for ic in range(JC):
            nc.sync.dma_start(ws_sb[:, ic, :, :], ins["ws"][ic])
        wi_sb = wpool.tile([128, MC, JC, 128], F32R, tag="wi")
        for mc in range(MC):
            nc.sync.dma_start(wi_sb[:, mc, :, :], ins["wi"][mc])
        wo_sb = wpool.tile([128, JC, MC, 128], F32R, tag="wo")
        for jc in range(JC):
            nc.sync.dma_start(wo_sb[:, jc, :, :], ins["wo"][jc])
        bias_sb = wpool.tile([128, JC, 1], F32, tag="bias")
        for jc in range(JC):
            nc.sync.dma_start(bias_sb[:, jc, :], ins["bias"][jc])

        # ---- row tiles, processed in interleaved pairs ----
        for tp in range(ntiles // 2):
            ctx = []
            for t in (2 * tp, 2 * tp + 1):
                xt = xpool.tile([128, MC, TILE_R], F32R, tag="xt")
                for mc in range(MC):
                    nc.sync.dma_start(
                        xt[:, mc, :], ins["xt"][mc, :, bass.ts(t, TILE_R)]
                    )
                zps = zpool.tile([128, JC, TILE_R], F32, tag="z")
                ctx.append(dict(t=t, xt=xt, zps=zps))

            # input projection: c = x @ W_in.T
            for d in ctx:
                zps = d["zps"]
                for jc in range(JC):
                    for mc in range(MC):
                        _mm(
                            nc,
                            zps[:, jc, :],
                            wi_sb[:, mc, jc, :],
                            d["xt"][:, mc, :],
                            start=(mc == 0),
                            stop=(mc == MC - 1),
                        )

            # keep c in SBUF; iteration 1: a_1 = tanh(c + bias)  (a_0 = 0)
            for d in ctx:
                c_sb = cpool.tile([128, JC, TILE_R], F32, tag="c")
                a = apool.tile([128, JC, TILE_R], F32R, tag="a")
                for jc in range(JC):
                    nc.vector.tensor_copy(c_sb[:, jc, :], d["zps"][:, jc, :])
                    nc.scalar.activation(
                        a[:, jc, :], d["zps"][:, jc, :], TANH,
                        bias=bias_sb[:, jc, :],
                    )
                d["c"] = c_sb
                d["a"] = a

            # iterations 2..15
            for k in range(1, K_ITERS):
                for d in ctx:
                    zps, a = d["zps"], d["a"]
                    for ic in range(JC):
                        for jc in range(JC):
                            _mm(
                                nc,
                                zps[:, jc, :],
                                ws_sb[:, ic, jc, :],
                                a[:, ic, :],
                                start=(ic == 0),
                                stop=(ic == JC - 1),
                            )
                for d in ctx:
                    t_sb = tpool.tile([128, JC, TILE_R], F32, tag="t")
                    a_new = apool.tile([128, JC, TILE_R], F32R, tag="a")
                    for jc in range(JC):
                        nc.vector.tensor_add(
                            t_sb[:, jc, :], d["zps"][:, jc, :], d["c"][:, jc, :]
                        )
                        nc.scalar.activation(
                            a_new[:, jc, :], t_sb[:, jc, :], TANH,
                            bias=bias_sb[:, jc, :],
                        )
                    d["a"] = a_new

            # output projection: yT = W_out @ a, reusing the first MC banks
            # of the (now closed) z PSUM tile.
            for d in ctx:
                zps = d["zps"]
                for mc in range(MC):
                    for jc in range(JC):
                        _mm(
                            nc,
                            zps[:, mc, :],
                            wo_sb[:, jc, mc, :],
                            d["a"][:, jc, :],
                            start=(jc == 0),
                            stop=(jc == JC - 1),
                        )
            for d in ctx:
                y_sb = ypool.tile([128, MC, TILE_R], F32, tag="y")
                for mc in range(MC):
                    nc.vector.tensor_copy(y_sb[:, mc, :], d["zps"][:, mc, :])
                    nc.sync.dma_start(
                        yt[mc, :, bass.ts(d["t"], TILE_R)], y_sb[:, mc, :]
                    )


def build_program(r_core=R_CORE, enable_asserts=False):
    nc = bacc.Bacc(
        "TRN2",
        target_bir_lowering=False,
        debug=False,
        enable_asserts=enable_asserts,
        num_devices=N_CORES,
        enable_partition_id=False,
    )
    ins = {
        "xt": nc.dram_tensor(
            "xt", [MC, 128, r_core], F32R, kind="ExternalInput"
        ).ap(),
        "ws": nc.dram_tensor(
            "ws", [JC, 128, JC, 128], F32R, kind="ExternalInput"
        ).ap(),
        "wi": nc.dram_tensor(
            "wi", [MC, 128, JC, 128], F32R, kind="ExternalInput"
        ).ap(),
        "wo": nc.dram_tensor(
            "wo", [JC, 128, MC, 128], F32R, kind="ExternalInput"
        ).ap(),
        "bias": nc.dram_tensor(
            "bias", [JC, 128, 1], F32, kind="ExternalInput"
        ).ap(),
    }
    yt = nc.dram_tensor(
        "yt", [MC, 128, r_core], F32, kind="ExternalOutput"
    ).ap()

    with tile.TileContext(nc) as tc:
        _body(tc, ins, yt, r_core)
    nc.compile()
    return nc


def prep_in_maps(x, W_in, b_in, W, b, W_out, b_out, r_core=R_CORE, n_cores=N_CORES):
    """Host-side packing: weight transposes + per-core transposed x shards."""
    x = np.ascontiguousarray(np.asarray(x, np.float32)).reshape(-1, C)
    W_in = np.asarray(W_in, np.float32)
    W = np.asarray(W, np.float32)
    W_out = np.asarray(W_out, np.float32)

    Ws = 0.5 * (W + W.T)
    shared = {
        "ws": np.ascontiguousarray(Ws.reshape(JC, 128, JC, 128)),
        "wi": np.ascontiguousarray(W_in.T.reshape(MC, 128, JC, 128)),
        "wo": np.ascontiguousarray(W_out.T.reshape(JC, 128, MC, 128)),
        "bias": np.ascontiguousarray(
            (np.asarray(b, np.float32) + np.asarray(b_in, np.float32)).reshape(
                JC, 128, 1
            )
        ),
    }
    in_maps = []
    for core in range(n_cores):
        xt = np.ascontiguousarray(x[core * r_core : (core + 1) * r_core].T)
        m = dict(shared)
        m["xt"] = xt.reshape(MC, 128, r_core)
        in_maps.append(m)
    return in_maps


def assemble_output(results, b_out, r_core=R_CORE):
    """results: list of per-core {"yt": [MC,128,r_core] f32} -> [B,L,C]."""
    parts = []
    for res in results:
        yt = np.asarray(res["yt"], np.float32).reshape(C, r_core)
        parts.append(yt.T)
    y = np.concatenate(parts, axis=0)
    y = y + np.asarray(b_out, np.float32)[None, :]
    if y.shape[0] == R_TOT:
        y = y.reshape(B, L, C)
    return np.ascontiguousarray(y.astype(np.float32))


_PROGRAM = None


def get_program():
    global _PROGRAM
    if _PROGRAM is None:
        _PROGRAM = build_program()
    return _PROGRAM


def run(inputs, trace=False, trace_kwargs=None):
    """Compile (cached) + execute on 8 cores; returns BassKernelResults."""
    nc = get_program()
    in_maps = prep_in_maps(**inputs)
    res = bass_utils.run_bass_kernel_spmd(
        nc,
        in_maps,
        core_ids=list(range(N_CORES)),
        trace=trace,
        **(trace_kwargs or {}),
    )
    return res


def kernel(x, W_in, b_in, W, b, W_out, b_out):
    inputs = dict(
        x=x, W_in=W_in, b_in=b_in, W=W, b=b, W_out=W_out, b_out=b_out
    )
    res = run(inputs, trace=False)
    return assemble_output(res.results, b_out)
